# revision 38
# baseline (speedup 1.0000x reference)
"""BertLayer forward on 8 Trainium2 NeuronCores.

Sharding: token-parallel compute (512 tokens/core, 4 cores per batch) with
on-device weight distribution. Host uploads are minimized: each core receives
only its own 512-token hidden slice and a 1/8 row-shard of each weight matrix,
all in bf16. Inside the kernel the weight shards are AllGathered across all 8
cores, and each core's locally-computed QKV projection is AllGathered across
its 4-core batch group in pipelined chunks that overlap the attention loop.

Numerics: all matmuls run in bf16 (1 cycle/row on the PE, fp32 PSUM
accumulation). Residual sums, LayerNorm statistics and softmax normalization
are carried in fp32.

Tricks (inherited from the tuned single-pass kernel):
  - attention mask folded into the scores matmul as a 65th contraction row
    (kT_aug row 64 = 8*mask[t], qT_aug row 64 = ones).
  - softmax denominators come free as a 65th output row of probs.T @ v_aug.
  - LayerNorm channel reductions are ones-vector matmuls on the PE;
    per-token mean/rstd rows are partition-broadcast by the GPSIMD engine.
"""
import numpy as np
import ml_dtypes
from contextlib import ExitStack

B, S, D = 2, 2048, 1024
H, DH = 16, 64
DFF = 4096
EPS = 1e-5
NCORES = 8
TOK = (B * S) // NCORES          # 512 tokens owned per core
CPB = NCORES // B                # 4 cores per batch
CH_T = D // 128                  # 8 channel tiles
DFF_T = DFF // 128               # 32 dff tiles
T_T = S // 128                   # 16 key-token tiles
SH_R = 128 // NCORES             # 16 weight rows uploaded per core

_CACHE = {}


def _build():
    import concourse.bass as bass
    import concourse.tile as tile
    from concourse import bacc, mybir
    from concourse.masks import make_identity
    from concourse.tile import add_dep_helper as _dep

    F32 = mybir.dt.float32
    F32R = mybir.dt.float32r
    BF16 = mybir.dt.bfloat16
    AF = mybir.ActivationFunctionType
    OP = mybir.AluOpType

    nc = bacc.Bacc("TRN2", target_bir_lowering=False, debug=False,
                   num_devices=NCORES)

    h_own = nc.dram_tensor("h_own", [D, TOK], BF16, kind="ExternalInput").ap()
    mask8 = nc.dram_tensor("mask8", [1, S], BF16, kind="ExternalInput").ap()
    wq_sh = nc.dram_tensor("wq_sh", [SH_R, CH_T * D], BF16, kind="ExternalInput").ap()
    wso_sh = nc.dram_tensor("wso_sh", [SH_R, CH_T * D], BF16, kind="ExternalInput").ap()
    wi_sh = nc.dram_tensor("wi_sh", [SH_R, DFF_T * D], BF16, kind="ExternalInput").ap()
    wo_sh = nc.dram_tensor("wo_sh", [SH_R, CH_T * DFF], BF16, kind="ExternalInput").ap()
    qb = nc.dram_tensor("qb", [128, CH_T], F32, kind="ExternalInput").ap()
    sob = nc.dram_tensor("sob", [128, CH_T], F32, kind="ExternalInput").ap()
    ib = nc.dram_tensor("ib", [128, DFF_T], F32, kind="ExternalInput").ap()
    ob = nc.dram_tensor("ob", [128, CH_T], F32, kind="ExternalInput").ap()
    l1g = nc.dram_tensor("l1g", [128, CH_T], F32, kind="ExternalInput").ap()
    l1b = nc.dram_tensor("l1b", [128, CH_T], F32, kind="ExternalInput").ap()
    l2g = nc.dram_tensor("l2g", [128, CH_T], F32, kind="ExternalInput").ap()
    l2b = nc.dram_tensor("l2b", [128, CH_T], F32, kind="ExternalInput").ap()
    out = nc.dram_tensor("out", [TOK, D], BF16, kind="ExternalOutput").ap()

    QUADS = [[0, 1, 2, 3], [4, 5, 6, 7]]
    ALL = [list(range(NCORES))]

    with tile.TileContext(nc) as tc, ExitStack() as root:
        # ---------------- DRAM bounce buffers + weight AllGathers ---------
        dram = root.enter_context(tc.tile_pool(name="dram", bufs=1, space="DRAM"))
        wq_in = dram.tile([SH_R, CH_T * D], BF16, tag="wq_in")
        wso_in = dram.tile([SH_R, CH_T * D], BF16, tag="wso_in")
        wi_in = dram.tile([SH_R, DFF_T * D], BF16, tag="wi_in")
        wo_in = dram.tile([SH_R, CH_T * DFF], BF16, tag="wo_in")
        wq_full = dram.tile([128, CH_T * D], BF16, tag="wq_full", addr_space="Shared")
        wso_full = dram.tile([128, CH_T * D], BF16, tag="wso_full", addr_space="Shared")
        wi_full = dram.tile([128, DFF_T * D], BF16, tag="wi_full", addr_space="Shared")
        wo_full = dram.tile([128, CH_T * DFF], BF16, tag="wo_full", addr_space="Shared")
        # qkv AllGather pipeline chunks: small first chunks so the attention
        # loop can start as early as possible, larger tail chunks for
        # efficiency.
        CHUNKS = [(0, 1), (1, 1), (2, 3), (5, 3)]      # (first m, n tiles)
        qkv_loc = [dram.tile([128, ln * TOK], BF16, tag=f"qkv_loc{c}",
                             name=f"qkv_loc{c}")
                   for c, (m0, ln) in enumerate(CHUNKS)]
        qkv_full = [dram.tile([CPB * 128, ln * TOK], BF16, tag=f"qkv_full{c}",
                              name=f"qkv_full{c}")
                    for c, (m0, ln) in enumerate(CHUNKS)]

        # wq's bounce DMA + AllGather trigger go first on the gpsimd queue:
        # wq gates phase 1, everything else can wait.
        nc.gpsimd.dma_start(wq_in[:], wq_sh[:])
        cc_wq = nc.gpsimd.collective_compute(
            "AllGather", OP.bypass, replica_groups=ALL,
            ins=[wq_in.opt()], outs=[wq_full.opt()])
        for t, a in ((wso_in, wso_sh), (wi_in, wi_sh), (wo_in, wo_sh)):
            nc.gpsimd.dma_start(t[:], a[:])

        const = root.enter_context(tc.tile_pool(name="const", bufs=1))
        ones2_f = const.tile([128, 2], F32, tag="ones2f")
        nc.vector.memset(ones2_f[:], 1.0)
        ones_col = const.tile([128, 1], F32R, tag="onescol")
        nc.vector.tensor_copy(ones_col[:], ones2_f[:, 0:1])
        ones_row = const.tile([1, TOK], BF16, tag="onesrowb")
        nc.vector.memset(ones_row[:], 1.0)
        ones128_f = const.tile([1, 128], F32, tag="ones128f")
        nc.vector.memset(ones128_f[:], 1.0)
        ones128 = const.tile([1, 128], F32R, tag="ones128")
        nc.vector.tensor_copy(ones128[:], ones128_f[:])
        ident_f = const.tile([128, 128], F32, tag="identf")
        make_identity(nc, ident_f[:])
        ident_b = const.tile([128, 128], BF16, tag="identb")
        nc.vector.tensor_copy(ident_b[:], ident_f[:])

        bias_p = root.enter_context(tc.tile_pool(name="bias", bufs=1))
        qb_s = bias_p.tile([128, CH_T], F32, tag="qb")
        sob_s = bias_p.tile([128, CH_T], F32, tag="sob")
        ib_s = bias_p.tile([128, DFF_T], F32, tag="ib")
        ob_s = bias_p.tile([128, CH_T], F32, tag="ob")
        l1g_s = bias_p.tile([128, CH_T], F32, tag="l1g")
        l1b_s = bias_p.tile([128, CH_T], F32, tag="l1b")
        l2g_s = bias_p.tile([128, CH_T], F32, tag="l2g")
        l2b_s = bias_p.tile([128, CH_T], F32, tag="l2b")
        for t, a in ((qb_s, qb), (sob_s, sob), (ib_s, ib), (ob_s, ob),
                     (l1g_s, l1g), (l1b_s, l1b), (l2g_s, l2g), (l2b_s, l2b)):
            nc.sync.dma_start(t[:], a[:])

        # DVE scratch shared by LN phases
        scr = root.enter_context(tc.tile_pool(name="scratch", bufs=2))

        # long-lived activation tensors, opened in LIFO-compatible order
        xln_scope = ExitStack()
        xlnp = xln_scope.enter_context(tc.tile_pool(name="xln", bufs=1))
        xln = xlnp.tile([128, CH_T * TOK], BF16, tag="xln")

        # FFN1 pools open before the attention/SO scopes so the first wi
        # weight slab can prefetch during earlier phases instead of waiting
        # for the phase-3 pools to release at LN1 time.
        g_scope = ExitStack()
        gp = g_scope.enter_context(tc.tile_pool(name="g_p", bufs=1))
        g_sb = gp.tile([128, DFF_T * TOK], BF16, tag="g")
        wist_scope = ExitStack()
        wist_p = wist_scope.enter_context(tc.tile_pool(name="wist_p", bufs=3))

        hq_scope = ExitStack()
        hqp = hq_scope.enter_context(tc.tile_pool(name="hq", bufs=1))
        h_own_s = hqp.tile([128, CH_T * TOK], BF16, tag="hown")
        qkv_ownT = hqp.tile([128, CH_T * TOK], BF16, tag="qkvown")

        attn_scope = ExitStack()
        attnp = attn_scope.enter_context(tc.tile_pool(name="attn", bufs=1))
        attnT = attnp.tile([128, CH_T * TOK], BF16, tag="attnT")

        qkv_scope = ExitStack()
        qkvp = qkv_scope.enter_context(tc.tile_pool(name="qkvT", bufs=1))
        qkvT = qkvp.tile([128, CH_T * S], BF16, tag="qkvT")

        for m in range(CH_T):
            nc.sync.dma_start(h_own_s[:, m * TOK:(m + 1) * TOK],
                              h_own[m * 128:(m + 1) * 128, :])

        # ---------------- Phase 1: qkv_own = wq @ h_own, quad AllGather ----
        with tc.tile_pool(name="wqst_p", bufs=1) as wqst_p, \
             tc.tile_pool(name="ps_qkv", bufs=3, space="PSUM") as ps_qkv:
            wq_st = wqst_p.tile([128, CH_T * D], BF16, tag="wqst")
            nc.sync.dma_start(wq_st[:], wq_full[:])
            cc_qkv = []
            for cch, (m0, ln) in enumerate(CHUNKS):
                for j in range(ln):
                    m = m0 + j
                    ps = ps_qkv.tile([128, TOK], F32, tag="ps")
                    for k in range(CH_T):
                        nc.tensor.matmul(
                            ps[:], wq_st[:, m * D + k * 128:m * D + k * 128 + 128],
                            h_own_s[:, k * TOK:(k + 1) * TOK],
                            start=(k == 0), stop=(k == CH_T - 1))
                    nc.vector.tensor_scalar_add(
                        qkv_ownT[:, m * TOK:(m + 1) * TOK], ps[:],
                        qb_s[:, m:m + 1])
                nc.sync.dma_start(
                    qkv_loc[cch][:],
                    qkv_ownT[:, m0 * TOK:(m0 + ln) * TOK])
                cc_qkv.append(nc.gpsimd.collective_compute(
                    "AllGather", OP.bypass, replica_groups=QUADS,
                    ins=[qkv_loc[cch].opt()], outs=[qkv_full[cch].opt()]))
        # remaining weight gathers, in consumption order
        cc_wso = nc.gpsimd.collective_compute(
            "AllGather", OP.bypass, replica_groups=ALL,
            ins=[wso_in.opt()], outs=[wso_full.opt()])
        cc_wi = nc.gpsimd.collective_compute(
            "AllGather", OP.bypass, replica_groups=ALL,
            ins=[wi_in.opt()], outs=[wi_full.opt()])
        cc_wo = nc.gpsimd.collective_compute(
            "AllGather", OP.bypass, replica_groups=ALL,
            ins=[wo_in.opt()], outs=[wo_full.opt()])
        # force the collective-queue order: wq, qkv chunks, wso, wi, wo —
        # the scheduler would otherwise emit the ready-first weight gathers
        # ahead of the phase-1-gated qkv gathers.
        chain = [cc_wq] + cc_qkv + [cc_wso, cc_wi, cc_wo]
        for a, b in zip(chain[1:], chain[:-1]):
            _dep(a.ins, b.ins, sync=False,
                 reason="collective queue priority order")

        # ---------------- Phase 2: attention ------------------------------
        # gathered bf16 qkv chunks are imported just-in-time inside the m
        # loop so attention on chunk c overlaps the AllGather of chunk c+1.
        GROUPS = [(i * 2, 2) for i in range(8)]
        GW = 2
        with tc.tile_pool(name="qg_p", bufs=3) as qg_p, \
             tc.tile_pool(name="vA_p", bufs=2) as vA_p, \
             tc.tile_pool(name="ktaug_p", bufs=2) as kt_p, \
             tc.tile_pool(name="qtaug_p", bufs=2) as qt_p, \
             tc.tile_pool(name="ps_tr", bufs=2, space="PSUM") as ps_tr, \
             tc.tile_pool(name="ps_sc", bufs=2, space="PSUM") as ps_sc, \
             tc.tile_pool(name="ps_at", bufs=2, space="PSUM") as ps_at, \
             tc.tile_pool(name="probs_p", bufs=3) as probs_p, \
             tc.tile_pool(name="rec_p", bufs=2) as rec_p:
            CH_AT = {m0: c for c, (m0, ln) in enumerate(CHUNKS)}
            for m in range(CH_T):
                if m in CH_AT:
                    cch = CH_AT[m]
                    m0, ln = CHUNKS[cch]
                    qg = qg_p.tile([128, CPB * ln * TOK], BF16, tag="qg")
                    nc.sync.dma_start(
                        qg[:].rearrange("p (r t) -> p r t", r=CPB),
                        qkv_full[cch][:].rearrange("(r p) t -> p r t", p=128))
                    for r in range(CPB):
                        for j in range(ln):
                            mm = m0 + j
                            nc.vector.tensor_copy(
                                qkvT[:, mm * S + r * TOK:mm * S + (r + 1) * TOK],
                                qg[:, (r * ln + j) * TOK:(r * ln + j + 1) * TOK])
                # v for heads 2m, 2m+1: transpose qkvT chunk to token-major,
                # interleave a ones column per head for the softmax denom.
                vA = vA_p.tile([128, T_T * 130], BF16, tag="vA")
                for i in range(T_T):
                    pt = ps_tr.tile([128, 128], BF16, tag="pt")
                    nc.tensor.transpose(
                        pt[:], qkvT[:, m * S + i * 128:m * S + (i + 1) * 128],
                        ident_b[:])
                    dst = vA[:, i * 130:(i + 1) * 130].rearrange(
                        "p (g c) -> p g c", c=65)[:, :, 0:64]
                    src = pt[:].rearrange("p (g c) -> p g c", g=2)
                    nc.vector.tensor_copy(dst, src)
                    ones_dst = vA[:, i * 130:(i + 1) * 130].rearrange(
                        "p (g c) -> p g c", c=65)[:, :, 64:65]
                    nc.vector.tensor_copy(
                        ones_dst, ones2_f[:].rearrange("p (g c) -> p g c", c=1))
                for sub in range(2):
                    h0 = sub * 64
                    ktaug = kt_p.tile([65, S], BF16, tag="ktaug")
                    nc.vector.tensor_copy(
                        ktaug[0:64, :], qkvT[h0:h0 + 64, m * S:(m + 1) * S])
                    nc.sync.dma_start(ktaug[64:65, :], mask8[:])
                    qtaug = qt_p.tile([65, TOK], BF16, tag="qtaug")
                    nc.vector.tensor_copy(
                        qtaug[0:64, :],
                        qkv_ownT[h0:h0 + 64, m * TOK:(m + 1) * TOK])
                    nc.vector.tensor_copy(qtaug[64:65, :], ones_row[:])

                    pat = ps_at.tile([65, TOK], F32, tag="pat")
                    for g0, glen in GROUPS:
                        psc = ps_sc.tile([128, GW * 512], F32, tag="psc")
                        for j in range(glen):
                            i = g0 + j
                            nc.tensor.matmul(
                                psc[:, j * 512:(j + 1) * 512],
                                ktaug[:, i * 128:(i + 1) * 128], qtaug[:],
                                start=True, stop=True)
                        probs = probs_p.tile([128, GW * 512], BF16, tag="probs")
                        nc.scalar.activation(
                            probs[:, 0:glen * 512], psc[:, 0:glen * 512],
                            AF.Exp, scale=float(1.0 / np.sqrt(DH)))
                        for j in range(glen):
                            i = g0 + j
                            nc.tensor.matmul(
                                pat[:],
                                vA[:, i * 130 + sub * 65:i * 130 + sub * 65 + 65],
                                probs[:, j * 512:(j + 1) * 512],
                                start=(i == 0), stop=(i == T_T - 1))
                    rec = rec_p.tile([1, TOK], F32, tag="rec")
                    nc.vector.reciprocal(rec[:], pat[64:65, :])
                    recb = rec_p.tile([64, TOK], F32, tag="recb")
                    nc.gpsimd.partition_broadcast(recb[:], rec[:])
                    nc.vector.tensor_mul(
                        attnT[h0:h0 + 64, m * TOK:(m + 1) * TOK],
                        pat[0:64, :], recb[:])
        qkv_scope.close()

        # ---------------- Phase 3: self-output + LN1 ----------------------
        with tc.tile_pool(name="wsost_p", bufs=3) as wsost_p, \
             tc.tile_pool(name="x_p", bufs=1) as x_p, \
             tc.tile_pool(name="ps_so", bufs=3, space="PSUM") as ps_so, \
             tc.tile_pool(name="ps_sum", bufs=1, space="PSUM") as ps_sum, \
             tc.tile_pool(name="ln_small", bufs=1) as lnp, \
             tc.tile_pool(name="lnb_p", bufs=1, space="PSUM") as lnb_p:
            x_sb = x_p.tile([128, CH_T * TOK], F32R, tag="x")
            pss = ps_sum.tile([1, TOK], F32, tag="s")
            psq = ps_sum.tile([1, TOK], F32, tag="q")
            wso_st = [None, None]
            for half in range(2):
                wso_st[half] = wsost_p.tile([128, 4 * D], BF16, tag="wsost",
                                            name=f"wso_st{half}")
                nc.sync.dma_start(wso_st[half][:],
                                  wso_full[:, half * 4 * D:(half + 1) * 4 * D])
            for m in range(CH_T):
                wst = wso_st[m // 4]
                mo = (m % 4) * D
                ps = ps_so.tile([128, TOK], F32, tag="ps")
                for k in range(CH_T):
                    nc.tensor.matmul(
                        ps[:], wst[:, mo + k * 128:mo + (k + 1) * 128],
                        attnT[:, k * TOK:(k + 1) * TOK],
                        start=(k == 0), stop=(k == CH_T - 1))
                xs = x_sb[:, m * TOK:(m + 1) * TOK]
                nc.vector.scalar_tensor_tensor(
                    xs, ps[:], sob_s[:, m:m + 1],
                    h_own_s[:, m * TOK:(m + 1) * TOK], OP.add, OP.add)
                sq = scr.tile([128, TOK], F32R, tag="sq")
                nc.vector.tensor_mul(sq[:], xs, xs)
                nc.tensor.matmul(pss[:], ones_col[:], xs,
                                 start=(m == 0), stop=(m == CH_T - 1))
                nc.tensor.matmul(psq[:], ones_col[:], sq[:],
                                 start=(m == 0), stop=(m == CH_T - 1))

            mu = lnp.tile([1, TOK], F32R, tag="mu1")
            ex2 = lnp.tile([1, TOK], F32, tag="ex21")
            nc.scalar.mul(mu[:], pss[:], 1.0 / D)
            nc.scalar.mul(ex2[:], psq[:], 1.0 / D)
            sqmu = lnp.tile([1, TOK], F32, tag="sqmu1")
            nc.vector.tensor_mul(sqmu[:], mu[:], mu[:])
            vare = lnp.tile([1, TOK], F32, tag="vare1")
            nc.vector.scalar_tensor_tensor(vare[:], ex2[:], EPS, sqmu[:],
                                           OP.add, OP.subtract)
            rcp = lnp.tile([1, TOK], F32, tag="rcp1")
            nc.vector.reciprocal(rcp[:], vare[:])
            rstd = lnp.tile([1, TOK], F32R, tag="rstd1")
            nc.scalar.sqrt(rstd[:], rcp[:])
            # broadcast mean/rstd across partitions with K=1 ones-matmuls on
            # the (idle) PE instead of the slower gpsimd partition_broadcast
            rstd_b = lnb_p.tile([128, TOK], F32, tag="rstdb1")
            mu_b = lnb_p.tile([128, TOK], F32, tag="mub1")
            nc.tensor.matmul(rstd_b[:], ones128[:], rstd[:],
                             start=True, stop=True)
            nc.tensor.matmul(mu_b[:], ones128[:], mu[:], start=True, stop=True)
            for m in range(CH_T):
                xs = x_sb[:, m * TOK:(m + 1) * TOK]
                d = scr.tile([128, TOK], F32, tag="d")
                nc.vector.tensor_sub(d[:], xs, mu_b[:])
                e = scr.tile([128, TOK], F32, tag="e")
                nc.vector.scalar_tensor_tensor(
                    e[:], d[:], l1g_s[:, m:m + 1], rstd_b[:], OP.mult, OP.mult)
                nc.scalar.activation(
                    xln[:, m * TOK:(m + 1) * TOK], e[:], AF.Identity,
                    bias=l1b_s[:, m:m + 1])
        attn_scope.close()
        hq_scope.close()

        # ---------------- Phase 4: FFN1 + GELU ----------------------------
        with tc.tile_pool(name="ps_f1", bufs=3, space="PSUM") as ps_f1:
            # weight slabs are fetched 4-at-a-time (1 MiB per DMA): per-slab
            # DMAs have ~2us fixed cost each, which starves the PE and lets
            # the HAM clock-gate re-throttle it to 1.2 GHz.
            for m4 in range(DFF_T // 4):
                wi_st = wist_p.tile([128, 4 * D], BF16, tag="wist")
                nc.sync.dma_start(
                    wi_st[:], wi_full[:, m4 * 4 * D:(m4 + 1) * 4 * D])
                for mm in range(4):
                    m = m4 * 4 + mm
                    ps = ps_f1.tile([128, TOK], F32, tag="ps")
                    for k in range(CH_T):
                        nc.tensor.matmul(
                            ps[:], wi_st[:, mm * D + k * 128:mm * D + (k + 1) * 128],
                            xln[:, k * TOK:(k + 1) * TOK],
                            start=(k == 0), stop=(k == CH_T - 1))
                    nc.scalar.activation(
                        g_sb[:, m * TOK:(m + 1) * TOK], ps[:],
                        AF.Gelu, bias=ib_s[:, m:m + 1])
        wist_scope.close()

        # ---------------- Phase 5: FFN2 + LN2 + transpose out -------------
        with tc.tile_pool(name="wost_p", bufs=2) as wost_p, \
             tc.tile_pool(name="ps_f2", bufs=2, space="PSUM") as ps_f2, \
             tc.tile_pool(name="z_p", bufs=1) as z_p, \
             tc.tile_pool(name="ps_sum2", bufs=1, space="PSUM") as ps_sum2, \
             tc.tile_pool(name="ln2_small", bufs=1) as ln2p, \
             tc.tile_pool(name="ln2b_p", bufs=1, space="PSUM") as ln2b_p, \
             tc.tile_pool(name="y_p", bufs=2) as y_p, \
             tc.tile_pool(name="ps_otr", bufs=2, space="PSUM") as ps_otr, \
             tc.tile_pool(name="stage_p", bufs=1) as stage_p:
            z_sb = z_p.tile([128, CH_T * TOK], F32R, tag="z")
            pss2 = ps_sum2.tile([1, TOK], F32, tag="s")
            psq2 = ps_sum2.tile([1, TOK], F32, tag="q")
            for m in range(CH_T):
                wo_st = wost_p.tile([128, DFF], BF16, tag="wost")
                nc.sync.dma_start(wo_st[:], wo_full[:, m * DFF:(m + 1) * DFF])
                ps = ps_f2.tile([128, TOK], F32, tag="ps")
                for k in range(DFF_T):
                    nc.tensor.matmul(
                        ps[:], wo_st[:, k * 128:(k + 1) * 128],
                        g_sb[:, k * TOK:(k + 1) * TOK],
                        start=(k == 0), stop=(k == DFF_T - 1))
                zs = z_sb[:, m * TOK:(m + 1) * TOK]
                nc.vector.scalar_tensor_tensor(
                    zs, ps[:], ob_s[:, m:m + 1],
                    xln[:, m * TOK:(m + 1) * TOK], OP.add, OP.add)
                sq = scr.tile([128, TOK], F32R, tag="sq")
                nc.vector.tensor_mul(sq[:], zs, zs)
                nc.tensor.matmul(pss2[:], ones_col[:], zs,
                                 start=(m == 0), stop=(m == CH_T - 1))
                nc.tensor.matmul(psq2[:], ones_col[:], sq[:],
                                 start=(m == 0), stop=(m == CH_T - 1))

            mu2 = ln2p.tile([1, TOK], F32R, tag="mu2")
            ex22 = ln2p.tile([1, TOK], F32, tag="ex22")
            nc.scalar.mul(mu2[:], pss2[:], 1.0 / D)
            nc.scalar.mul(ex22[:], psq2[:], 1.0 / D)
            sqmu2 = ln2p.tile([1, TOK], F32, tag="sqmu2")
            nc.vector.tensor_mul(sqmu2[:], mu2[:], mu2[:])
            vare2 = ln2p.tile([1, TOK], F32, tag="vare2")
            nc.vector.scalar_tensor_tensor(vare2[:], ex22[:], EPS, sqmu2[:],
                                           OP.add, OP.subtract)
            rcp2 = ln2p.tile([1, TOK], F32, tag="rcp2")
            nc.vector.reciprocal(rcp2[:], vare2[:])
            rstd2 = ln2p.tile([1, TOK], F32R, tag="rstd2")
            nc.scalar.sqrt(rstd2[:], rcp2[:])
            rstd2_b = ln2b_p.tile([128, TOK], F32, tag="rstdb2")
            mu2_b = ln2b_p.tile([128, TOK], F32, tag="mub2")
            nc.tensor.matmul(rstd2_b[:], ones128[:], rstd2[:],
                             start=True, stop=True)
            nc.tensor.matmul(mu2_b[:], ones128[:], mu2[:],
                             start=True, stop=True)

            stage = stage_p.tile([128, (TOK // 128) * D], BF16, tag="stage")
            for m in range(CH_T):
                zs = z_sb[:, m * TOK:(m + 1) * TOK]
                d = scr.tile([128, TOK], F32, tag="d")
                nc.vector.tensor_sub(d[:], zs, mu2_b[:])
                e = scr.tile([128, TOK], F32, tag="e")
                nc.vector.scalar_tensor_tensor(
                    e[:], d[:], l2g_s[:, m:m + 1], rstd2_b[:], OP.mult, OP.mult)
                y_m = y_p.tile([128, TOK], BF16, tag="y")
                nc.scalar.activation(y_m[:], e[:], AF.Identity,
                                     bias=l2b_s[:, m:m + 1])
                for j in range(TOK // 128):
                    pt = ps_otr.tile([128, 128], BF16, tag="pt")
                    nc.tensor.transpose(
                        pt[:], y_m[:, j * 128:(j + 1) * 128], ident_b[:])
                    nc.scalar.copy(
                        stage[:, j * D + m * 128:j * D + (m + 1) * 128], pt[:])
            for j in range(TOK // 128):
                nc.sync.dma_start(out[j * 128:(j + 1) * 128, :],
                                  stage[:, j * D:(j + 1) * D])
        g_scope.close()
        xln_scope.close()
    nc.finalize()
    return nc


def _blockify(wt, kt, mt):
    # wt: [kt*128, mt*128] (already W.T). Block (m, k) lands at columns
    # [m*kt*128 + k*128, ...+128) so a per-m slab is one contiguous DMA.
    return np.ascontiguousarray(
        wt.reshape(kt, 128, mt, 128).transpose(1, 2, 0, 3).reshape(128, -1))


def _cols(bias, nt):
    return np.ascontiguousarray(np.asarray(bias, np.float32).reshape(nt, 128).T)


def _build_in_maps(hidden_state, attention_mask, q_w, q_b, so_w, so_b,
                   ln1_g, ln1_b, inter_w, inter_b, out_w, out_b, ln2_g, ln2_b):
    bf16 = ml_dtypes.bfloat16
    hidden_state = np.asarray(hidden_state, np.float32)
    attention_mask = np.asarray(attention_mask, np.float32)
    wq_b = _blockify(np.asarray(q_w, np.float32).T, CH_T, CH_T).astype(bf16)
    wso_b = _blockify(np.asarray(so_w, np.float32).T, CH_T, CH_T).astype(bf16)
    wi_b = _blockify(np.asarray(inter_w, np.float32).T, CH_T, DFF_T).astype(bf16)
    wo_b = _blockify(np.asarray(out_w, np.float32).T, DFF_T, CH_T).astype(bf16)
    shared = {
        "qb": _cols(q_b, CH_T), "sob": _cols(so_b, CH_T),
        "ib": _cols(inter_b, DFF_T), "ob": _cols(out_b, CH_T),
        "l1g": _cols(ln1_g, CH_T), "l1b": _cols(ln1_b, CH_T),
        "l2g": _cols(ln2_g, CH_T), "l2b": _cols(ln2_b, CH_T),
    }
    in_maps = []
    for c in range(NCORES):
        b, r = divmod(c, CPB)
        ht = hidden_state[b].T                               # [D, S]
        m8 = (8.0 * attention_mask[b, 0, 0, :]).reshape(1, S)
        sh = slice(SH_R * c, SH_R * (c + 1))
        in_maps.append({
            **shared,
            "h_own": np.ascontiguousarray(
                ht[:, r * TOK:(r + 1) * TOK].astype(bf16)),
            "mask8": np.ascontiguousarray(m8.astype(bf16)),
            "wq_sh": np.ascontiguousarray(wq_b[sh]),
            "wso_sh": np.ascontiguousarray(wso_b[sh]),
            "wi_sh": np.ascontiguousarray(wi_b[sh]),
            "wo_sh": np.ascontiguousarray(wo_b[sh]),
        })
    return in_maps


def kernel(hidden_state, attention_mask, q_w, q_b, so_w, so_b, ln1_g, ln1_b,
           inter_w, inter_b, out_w, out_b, ln2_g, ln2_b):
    from concourse.bass_utils import run_bass_kernel_spmd

    if "nc" not in _CACHE:
        _CACHE["nc"] = _build()
    nc = _CACHE["nc"]

    in_maps = _build_in_maps(
        hidden_state, attention_mask, q_w, q_b, so_w, so_b, ln1_g, ln1_b,
        inter_w, inter_b, out_w, out_b, ln2_g, ln2_b)

    res = run_bass_kernel_spmd(nc, in_maps, list(range(NCORES)))
    full = np.empty((B, S, D), np.float32)
    for c in range(NCORES):
        b, r = divmod(c, CPB)
        full[b, r * TOK:(r + 1) * TOK, :] = \
            res.results[c]["out"].astype(np.float32)
    return full
